# revision 1
# baseline (speedup 1.0000x reference)
"""Multi-head attention (B=2, S=2048, H=1024, 16 heads x 64) on 8 TRN2 cores.

Sharding: data-parallel over batch (cores 0-3 -> b=0, cores 4-7 -> b=1),
tensor-parallel over heads (4 heads / core, i.e. a 256-column slice of
Wq/Wk/Wv).  Each core computes its heads' full attention; the host
assembles the [B, S, 1024] output from the per-core [S, 256] slices.

Per-core kernel layout (all on-chip, no DRAM spill of scores):
  - QT/KT stored as [head_dim(c), seq] so stage A computes S^T tiles
    [j, i] directly; no max-subtraction is needed (scores are O(1) and
    masked entries underflow to exactly 0 after exp).
  - mask applied multiplicatively after exp:  exp(s/8 - 10000*m)
    == exp(s/8) * (1-m)  exactly in fp32 for m in {0,1}.
  - V is augmented with a ones column, so stage B's matmul also yields
    the softmax denominator for free; a tiny PE transpose at the end
    moves [65, i] tiles to [i, 65] where the divide is per-partition.
  - projections + stage A run in fp32r (full PE rate, ~1e-4 rel err),
    probs/V in bf16 (stage B), everything accumulated in fp32.
"""

import sys

if "/opt/trn_rl_repo" not in sys.path:
    sys.path.insert(0, "/opt/trn_rl_repo")

from contextlib import ExitStack

import ml_dtypes
import numpy as np

import concourse.bass as bass
import concourse.tile as tile
from concourse import bacc, mybir
from concourse.bass_utils import run_bass_kernel_spmd
from concourse.masks import make_identity

F32 = mybir.dt.float32
F32R = mybir.dt.float32r
BF16 = mybir.dt.bfloat16
FP16 = mybir.dt.float16

B, S, H = 2, 2048, 1024
NH, HD = 16, 64
NCORES = 8
CORES_PER_B = NCORES // B          # 4
LHEADS = NH // CORES_PER_B         # 4 heads per core
COLS = LHEADS * HD                 # 256 projection columns per core
HC = H // 128                      # 8 contraction chunks
SBLK = 512                         # seq block (phase 1 moving dim / i-block)
NSB = S // SBLK                    # 4
JT = S // 128                      # 16 j tiles
JG = JT // 2                       # 8 groups of 2 j-tiles (ACT FD=1024)


def build_program(reps=1, interleave_b=False, sc_fd512=False, p4=True, gsplit=0, zz=False, single_pass=False, fp16=False, mbufs=2, xbufs=None):
    WDT = FP16 if fp16 else F32R
    PDT = FP16 if fp16 else BF16
    nc = bacc.Bacc("TRN2", target_bir_lowering=False, debug=False)

    xT = nc.dram_tensor("xT", [H, S], WDT, kind="ExternalInput").ap()
    maskp = nc.dram_tensor("maskp", [S, S], PDT, kind="ExternalInput").ap()
    wq = nc.dram_tensor("wq", [H, COLS], WDT, kind="ExternalInput").ap()
    wk = nc.dram_tensor("wk", [H, COLS], WDT, kind="ExternalInput").ap()
    wv = nc.dram_tensor("wv", [H, COLS], WDT, kind="ExternalInput").ap()
    bq = nc.dram_tensor("bq", [COLS, 1], F32, kind="ExternalInput").ap()
    bk = nc.dram_tensor("bk", [COLS, 1], F32, kind="ExternalInput").ap()
    bv = nc.dram_tensor("bv", [1, COLS], WDT, kind="ExternalInput").ap()
    ones_d = nc.dram_tensor("ones_d", [1, 128], WDT, kind="ExternalInput").ap()
    out = nc.dram_tensor("out", [S, COLS], F32, kind="ExternalOutput").ap()

    with tile.TileContext(nc) as tc:
      for _rep in range(reps):
        with ExitStack() as ctx:
            persist = ctx.enter_context(tc.tile_pool(name="persist", bufs=1))
            wpool = ctx.enter_context(tc.tile_pool(name="wpool", bufs=1))
            xpool = ctx.enter_context(
                tc.tile_pool(
                    name="xpool",
                    bufs=(xbufs if xbufs else (2 if p4 else 3)),
                )
            )
            mpool = ctx.enter_context(tc.tile_pool(name="mpool", bufs=mbufs))
            ppool = ctx.enter_context(tc.tile_pool(name="ppool", bufs=1))
            opool = ctx.enter_context(tc.tile_pool(name="opool", bufs=2))
            upool = ctx.enter_context(tc.tile_pool(name="upool", bufs=2))
            rpool = ctx.enter_context(tc.tile_pool(name="rpool", bufs=2))
            psum = ctx.enter_context(tc.tile_pool(name="psum", bufs=1, space="PSUM"))

            # ---- constants / persistent ----
            ident = persist.tile([128, 128], F32)
            make_identity(nc, ident[:])
            ones1 = persist.tile([1, 128], WDT)
            bq_sb = persist.tile([128, 2], F32)
            bk_sb = persist.tile([128, 2], F32)
            bv_sb = persist.tile([1, COLS], WDT)

            def load_consts():
                nc.sync.dma_start(ones1[:], ones_d[:])
                for hp in range(2):
                    nc.sync.dma_start(
                        bq_sb[:, hp : hp + 1], bq[hp * 128 : hp * 128 + 128, :]
                    )
                    nc.sync.dma_start(
                        bk_sb[:, hp : hp + 1], bk[hp * 128 : hp * 128 + 128, :]
                    )
                nc.sync.dma_start(bv_sb[:], bv[:])

            QT = [persist.tile([128, S], WDT, name=f"QT{p}") for p in range(2)]
            KT = [persist.tile([128, S], WDT, name=f"KT{p}") for p in range(2)]
            Vp = persist.tile([128, JT, LHEADS, 66], PDT)
            nc.gpsimd.memset(Vp[:, :, :, 64:65], 1.0)

            # weights: K is loaded interleaved with the first xT block so the
            # first matmul starts after ~2 chunks; Q/V weights load later.
            wk_sb = wpool.tile([128, HC, COLS], WDT)
            wq_sb = wpool.tile([128, HC, COLS], WDT)
            wv_sb = wpool.tile([128, HC, COLS], WDT)

            def load_w(w_sb, w_dram):
                for hc in range(HC):
                    nc.sync.dma_start(
                        w_sb[:, hc, :], w_dram[hc * 128 : (hc + 1) * 128, :]
                    )

            def load_xt(sb):
                xt = xpool.tile([128, HC, SBLK], WDT, name="xt", tag="xt")
                for hc in range(HC):
                    nc.sync.dma_start(
                        xt[:, hc, :],
                        xT[hc * 128 : (hc + 1) * 128, sb * SBLK : (sb + 1) * SBLK],
                    )
                return xt

            def proj_k(sb, xt):
                sl = slice(sb * SBLK, (sb + 1) * SBLK)
                for hp in range(2):
                    cs = slice(hp * 128, hp * 128 + 128)
                    # pk borrows the scA/scB slots (attention only starts
                    # after K is fully projected anyway)
                    pk = psum.tile(
                        [128, SBLK], F32, name="pk", tag=f"sc{hp}",
                        bufs=(2 if sc_fd512 else 1),
                    )
                    for hc in range(HC):
                        nc.tensor.matmul(
                            pk[:], wk_sb[:, hc, cs], xt[:, hc, :],
                            start=(hc == 0), stop=(hc == HC - 1),
                        )
                    nc.vector.tensor_scalar_add(
                        KT[hp][:, sl], pk[:], bk_sb[:, hp : hp + 1]
                    )

            def proj_q(sb, xt):
                sl = slice(sb * SBLK, (sb + 1) * SBLK)
                for hp in range(2):
                    cs = slice(hp * 128, hp * 128 + 128)
                    pq = psum.tile([128, SBLK], F32, name="pq", tag="pq")
                    for hc in range(HC):
                        nc.tensor.matmul(
                            pq[:], wq_sb[:, hc, cs], xt[:, hc, :],
                            start=(hc == 0), stop=(hc == HC - 1),
                        )
                    nc.vector.tensor_scalar_add(
                        QT[hp][:, sl], pq[:], bq_sb[:, hp : hp + 1]
                    )

            def proj_v(sb, xt):
                for st4 in range(4):
                    st = sb * 4 + st4
                    pv = psum.tile(
                        [128, COLS], F32, name="pv", tag=f"po{st % 2}"
                    )
                    for hc in range(HC):
                        nc.tensor.matmul(
                            pv[:], xt[:, hc, st4 * 128 : st4 * 128 + 128],
                            wv_sb[:, hc, :],
                            start=(hc == 0), stop=False,
                        )
                    nc.tensor.matmul(
                        pv[:], ones1[:], bv_sb[:], start=False, stop=True
                    )
                    nc.vector.tensor_copy(
                        Vp[:, st, :, 0:64],
                        pv.rearrange("p (h d) -> p h d", h=LHEADS),
                    )

            # K pass first (attention stage A gates on full KT), then Q/V.
            # First block: interleave wk-chunk and xt-chunk DMAs so the
            # leading matmul's operands land after ~400KB of traffic.
            xt0 = xpool.tile([128, HC, SBLK], WDT, name="xt", tag="xt")
            for hc in range(HC):
                nc.sync.dma_start(
                    wk_sb[:, hc, :], wk[hc * 128 : (hc + 1) * 128, :]
                )
                nc.sync.dma_start(
                    xt0[:, hc, :], xT[hc * 128 : (hc + 1) * 128, 0:SBLK]
                )
            load_consts()
            if single_pass:
                load_w(wq_sb, wq)
                load_w(wv_sb, wv)
                proj_k(0, xt0)
                proj_q(0, xt0)
                proj_v(0, xt0)
                for sb in range(1, NSB):
                    xt = load_xt(sb)
                    proj_k(sb, xt)
                    proj_q(sb, xt)
                    proj_v(sb, xt)
            else:
                proj_k(0, xt0)
                for sb in range(1, NSB):
                    xt = load_xt(sb)
                    if sb == 1:
                        load_w(wq_sb, wq)
                    elif sb == 2:
                        load_w(wv_sb, wv)
                    proj_k(sb, xt)
                for sb in range(NSB):
                    xt = load_xt(sb)
                    proj_q(sb, xt)
                    proj_v(sb, xt)

            # ---- attention ----
            if zz:
                # software-pipelined: P split into 8 half-buffers
                # (hp x hl x half); stage A/B zig-zag so ACT (exp) never
                # starves while PE runs stage B / projections.
                HJT = JT // 2  # j-tiles per half

                def emit_A(ib, hp, half, Ph):
                    isl = slice(ib * SBLK, (ib + 1) * SBLK)
                    mt = mts[ib]
                    for g in range(HJT // 2):
                        for hl in range(2):
                            ps = psum.tile(
                                [128, 2, SBLK], F32, name=f"sc{hl}", tag=f"sc{hl}"
                            )
                            rows = slice(hl * 64, hl * 64 + 64)
                            for jj in range(2):
                                jt_ = half * HJT + g * 2 + jj
                                nc.tensor.matmul(
                                    ps[:, jj, :],
                                    KT[hp][rows, jt_ * 128 : jt_ * 128 + 128],
                                    QT[hp][rows, isl],
                                    start=True,
                                    stop=True,
                                )
                            gsl = slice(g * 2, g * 2 + 2)
                            msl = slice(half * HJT + g * 2, half * HJT + g * 2 + 2)
                            nc.scalar.activation(
                                Ph[hl][:, gsl, :],
                                ps[:],
                                mybir.ActivationFunctionType.Exp,
                                scale=0.125,
                            )
                            nc.vector.tensor_mul(
                                Ph[hl][:, gsl, :], Ph[hl][:, gsl, :], mt[:, msl, :]
                            )

                def emit_B(ib, hp, half, Ph, po2):
                    for hl in range(2):
                        h = hp * 2 + hl
                        for g in range(HJT):
                            jt_ = half * HJT + g
                            nc.tensor.matmul(
                                po2[hl][:],
                                Vp[:, jt_, h, 0:65],
                                Ph[hl][:, g, :],
                                start=(jt_ == 0),
                                stop=(jt_ == JT - 1),
                                skip_group_check=True,
                            )

                def emit_epi(ib, hp, outt, po2):
                    for hl in range(2):
                        h = hp * 2 + hl
                        po = po2[hl]
                        u = upool.tile([65, SBLK], F32, name="u")
                        nc.vector.tensor_copy(u[:], po[:])
                        pt = psum.tile([128, 4, 65], F32, name="pt", tag="pt")
                        for c in range(4):
                            nc.tensor.transpose(
                                pt[:, c, :],
                                u[:, c * 128 : (c + 1) * 128],
                                ident[0:65, 0:65],
                            )
                        rec = rpool.tile([128, 4], F32, name="rec")
                        nc.vector.reciprocal(rec[:], pt[:, :, 64])
                        for c in range(4):
                            nc.vector.tensor_scalar_mul(
                                outt[:, c, h * 64 : h * 64 + 64],
                                pt[:, c, 0:64],
                                rec[:, c : c + 1],
                            )

                def new_P(hp):
                    return [
                        [
                            ppool.tile(
                                [128, HJT, SBLK], PDT,
                                name=f"P{hp}{hl}{hf}", tag=f"P{hp}{hl}{hf}",
                            )
                            for hf in range(2)
                        ]
                        for hl in range(2)
                    ]

                def P_half(P, hf):
                    return [P[0][hf], P[1][hf]]

                mts = {}
                for ib in range(NSB):
                    isl = slice(ib * SBLK, (ib + 1) * SBLK)
                    mts[ib] = mpool.tile([128, JT, SBLK], PDT, name="mt")
                    nc.sync.dma_start(
                        mts[ib][:],
                        maskp[:, isl].rearrange("(t p) i -> p t i", p=128),
                    )
                    outt = opool.tile([128, 4, COLS], F32, name="outt")
                    P0 = new_P(0)
                    P1 = new_P(1)
                    po0 = [
                        psum.tile([65, SBLK], F32, name=f"po{hl}", tag=f"po{hl}")
                        for hl in range(2)
                    ]
                    emit_A(ib, 0, 0, P_half(P0, 0))
                    emit_A(ib, 0, 1, P_half(P0, 1))
                    emit_A(ib, 1, 0, P_half(P1, 0))
                    emit_B(ib, 0, 0, P_half(P0, 0), po0)
                    emit_A(ib, 1, 1, P_half(P1, 1))
                    emit_B(ib, 0, 1, P_half(P0, 1), po0)
                    emit_epi(ib, 0, outt, po0)
                    po1 = [
                        psum.tile([65, SBLK], F32, name=f"qo{hl}", tag=f"po{hl}")
                        for hl in range(2)
                    ]
                    emit_B(ib, 1, 0, P_half(P1, 0), po1)
                    emit_B(ib, 1, 1, P_half(P1, 1), po1)
                    emit_epi(ib, 1, outt, po1)
                    nc.sync.dma_start(
                        out[isl, :].rearrange("(c p) n -> p c n", p=128), outt[:]
                    )
            else:
                for ib in range(NSB):
                    isl = slice(ib * SBLK, (ib + 1) * SBLK)
                    mt = mpool.tile([128, JT, SBLK], PDT, name="mt")
                    nc.sync.dma_start(
                        mt[:], maskp[:, isl].rearrange("(t p) i -> p t i", p=128)
                    )
                    outt = opool.tile([128, 4, COLS], F32, name="outt")
                    for hp in range(2):
                        ptag = f"P{hp}" if p4 else "P"
                        P2h = [
                            ppool.tile(
                                [128, JT, SBLK], PDT, name=f"P{hl}",
                                tag=f"{ptag}{hl}",
                            )
                            for hl in range(2)
                        ]
                        po2 = [
                            psum.tile([65, SBLK], F32, name=f"po{hl}", tag=f"po{hl}")
                            for hl in range(2)
                        ]

                        def stage_b(jg_done):
                            for hl in range(2):
                                h = hp * 2 + hl
                                for jj in range(2):
                                    jt_ = jg_done * 2 + jj
                                    nc.tensor.matmul(
                                        po2[hl][:],
                                        Vp[:, jt_, h, 0:65],
                                        P2h[hl][:, jt_, :],
                                        start=(jt_ == 0),
                                        stop=(jt_ == JT - 1),
                                        skip_group_check=True,
                                    )

                        if sc_fd512:
                            for jt_ in range(JT):
                                for hl in range(2):
                                    ps = psum.tile(
                                        [128, SBLK], F32, name=f"sc{hl}",
                                        tag=f"sc{hl}", bufs=2,
                                    )
                                    rows = slice(hl * 64, hl * 64 + 64)
                                    nc.tensor.matmul(
                                        ps[:],
                                        KT[hp][rows, jt_ * 128 : jt_ * 128 + 128],
                                        QT[hp][rows, isl],
                                        start=True,
                                        stop=True,
                                    )
                                    nc.scalar.activation(
                                        P2h[hl][:, jt_, :],
                                        ps[:],
                                        mybir.ActivationFunctionType.Exp,
                                        scale=0.125,
                                    )
                                    nc.vector.tensor_mul(
                                        P2h[hl][:, jt_, :],
                                        P2h[hl][:, jt_, :],
                                        mt[:, jt_, :],
                                    )
                        else:
                          for jg in range(JG):
                            for hl in range(2):
                                ps = psum.tile(
                                    [128, 2, SBLK], F32, name=f"sc{hl}", tag=f"sc{hl}"
                                )
                                rows = slice(hl * 64, hl * 64 + 64)
                                for jj in range(2):
                                    jt_ = jg * 2 + jj
                                    nc.tensor.matmul(
                                        ps[:, jj, :],
                                        KT[hp][rows, jt_ * 128 : jt_ * 128 + 128],
                                        QT[hp][rows, isl],
                                        start=True,
                                        stop=True,
                                    )
                                gsl = slice(jg * 2, jg * 2 + 2)
                                nc.scalar.activation(
                                    P2h[hl][:, gsl, :],
                                    ps[:],
                                    mybir.ActivationFunctionType.Exp,
                                    scale=0.125,
                                )
                                eng = (
                                    nc.gpsimd
                                    if gsplit and (jg % gsplit == gsplit - 1)
                                    else nc.vector
                                )
                                eng.tensor_mul(
                                    P2h[hl][:, gsl, :], P2h[hl][:, gsl, :], mt[:, gsl, :]
                                )
                            if interleave_b and jg >= 1:
                                stage_b(jg - 1)
                        if interleave_b:
                            stage_b(JG - 1)
                        else:
                            for jg_ in range(JG):
                                stage_b(jg_)

                        for hl in range(2):
                            h = hp * 2 + hl
                            po = po2[hl]
                            u = upool.tile([65, SBLK], F32, name="u")
                            nc.vector.tensor_copy(u[:], po[:])
                            pt = psum.tile([128, 4, 65], F32, name="pt", tag="pt")
                            for c in range(4):
                                nc.tensor.transpose(
                                    pt[:, c, :],
                                    u[:, c * 128 : (c + 1) * 128],
                                    ident[0:65, 0:65],
                                )
                            rec = rpool.tile([128, 4], F32, name="rec")
                            nc.vector.reciprocal(rec[:], pt[:, :, 64])
                            for c in range(4):
                                nc.vector.tensor_scalar_mul(
                                    outt[:, c, h * 64 : h * 64 + 64],
                                    pt[:, c, 0:64],
                                    rec[:, c : c + 1],
                                )
                    nc.sync.dma_start(
                        out[isl, :].rearrange("(c p) n -> p c n", p=128), outt[:]
                    )

    nc.compile()
    return nc


_NC_CACHE = []


def get_nc():
    if not _NC_CACHE:
        _NC_CACHE.append(
            build_program(fp16=USE_FP16, single_pass=True, mbufs=3)
        )
    return _NC_CACHE[0]


def make_in_maps(x, attn_mask, Wq, bq, Wk, bk, Wv, bv, fp16=False):
    wdt = np.float16 if fp16 else np.float32
    pdt = np.float16 if fp16 else ml_dtypes.bfloat16
    x = np.asarray(x, dtype=np.float32)
    attn_mask = np.asarray(attn_mask)
    Wq, Wk, Wv = (np.asarray(w, dtype=np.float32) for w in (Wq, Wk, Wv))
    bq, bk, bv = (np.asarray(b_, dtype=np.float32) for b_ in (bq, bk, bv))

    in_maps = []
    for core in range(NCORES):
        b = core // CORES_PER_B
        hg = core % CORES_PER_B
        cs = slice(hg * COLS, (hg + 1) * COLS)
        mp = (1 - attn_mask[b].T).astype(pdt)
        in_maps.append(
            {
                "xT": np.ascontiguousarray(x[b].T.astype(wdt)),
                "maskp": np.ascontiguousarray(mp),
                "wq": np.ascontiguousarray(Wq[:, cs].astype(wdt)),
                "wk": np.ascontiguousarray(Wk[:, cs].astype(wdt)),
                "wv": np.ascontiguousarray(Wv[:, cs].astype(wdt)),
                "bq": np.ascontiguousarray(bq[cs, None]),
                "bk": np.ascontiguousarray(bk[cs, None]),
                "bv": np.ascontiguousarray(bv[None, cs].astype(wdt)),
                "ones_d": np.ones((1, 128), wdt),
            }
        )
    return in_maps


def assemble(results):
    out = np.empty((B, S, H), np.float32)
    for core in range(NCORES):
        b = core // CORES_PER_B
        hg = core % CORES_PER_B
        out[b, :, hg * COLS : (hg + 1) * COLS] = results[core]["out"]
    return out


USE_FP16 = True


def kernel(x, attn_mask, Wq, bq, Wk, bk, Wv, bv):
    nc = get_nc()
    in_maps = make_in_maps(x, attn_mask, Wq, bq, Wk, bk, Wv, bv, fp16=USE_FP16)
    res = run_bass_kernel_spmd(nc, in_maps, list(range(NCORES)))
    return assemble(res.results)



# revision 34
# speedup vs baseline: 1.0338x; 1.0338x over previous
"""Multi-head attention (B=2, S=2048, H=1024, 16 heads x 64) on 8 TRN2 cores.

Sharding: data-parallel over batch (cores 0-3 -> b=0, cores 4-7 -> b=1),
tensor-parallel over heads (4 heads / core, i.e. a 256-column slice of
Wq/Wk/Wv).  Each core computes its heads' full attention; the host
assembles the [B, S, 1024] output from the per-core [S, 256] slices.

Per-core kernel layout (all on-chip, no DRAM spill of scores):
  - QT/KT stored as [head_dim(c), seq] so stage A computes S^T tiles
    [j, i] directly; no max-subtraction is needed (scores are O(1) and
    masked entries underflow to exactly 0 after exp).
  - mask applied multiplicatively after exp:  exp(s/8 - 10000*m)
    == exp(s/8) * (1-m)  exactly in fp32 for m in {0,1}.
  - V is augmented with a ones column, so stage B's matmul also yields
    the softmax denominator for free; a tiny PE transpose at the end
    moves [65, i] tiles to [i, 65] where the divide is per-partition.
  - projections + stage A run in fp32r (full PE rate, ~1e-4 rel err),
    probs/V in bf16 (stage B), everything accumulated in fp32.
"""

import sys

if "/opt/trn_rl_repo" not in sys.path:
    sys.path.insert(0, "/opt/trn_rl_repo")

from contextlib import ExitStack

import ml_dtypes
import numpy as np

import concourse.bass as bass
import concourse.tile as tile
from concourse import bacc, mybir
from concourse.bass_utils import run_bass_kernel_spmd
from concourse.masks import make_identity

F32 = mybir.dt.float32
F32R = mybir.dt.float32r
BF16 = mybir.dt.bfloat16
FP16 = mybir.dt.float16

B, S, H = 2, 2048, 1024
NH, HD = 16, 64
NCORES = 8
CORES_PER_B = NCORES // B          # 4
LHEADS = NH // CORES_PER_B         # 4 heads per core
COLS = LHEADS * HD                 # 256 projection columns per core
HC = H // 128                      # 8 contraction chunks
SBLK = 512                         # seq block (phase 1 moving dim / i-block)
NSB = S // SBLK                    # 4
JT = S // 128                      # 16 j tiles
JG = JT // 2                       # 8 groups of 2 j-tiles (ACT FD=1024)


def build_program(reps=1, interleave_b=False, sc_fd512=False, p4=True, gsplit=0, zz=False, single_pass=False, fp16=False, mbufs=2, xbufs=None, sc3=False, dma2=False, early=False):
    WDT = FP16 if fp16 else F32R
    PDT = FP16 if fp16 else BF16
    nc = bacc.Bacc("TRN2", target_bir_lowering=False, debug=False)

    xT = nc.dram_tensor("xT", [H, S], WDT, kind="ExternalInput").ap()
    maskp = nc.dram_tensor("maskp", [S, S], PDT, kind="ExternalInput").ap()
    wq = nc.dram_tensor("wq", [H, COLS], WDT, kind="ExternalInput").ap()
    wk = nc.dram_tensor("wk", [H, COLS], WDT, kind="ExternalInput").ap()
    wv = nc.dram_tensor("wv", [H, COLS], WDT, kind="ExternalInput").ap()
    if early:
        bqk = nc.dram_tensor("bqk", [COLS, 2], F32, kind="ExternalInput").ap()
        cw = nc.dram_tensor("cw", [1, COLS + 128], WDT, kind="ExternalInput").ap()
    else:
        bq = nc.dram_tensor("bq", [COLS, 1], F32, kind="ExternalInput").ap()
        bk = nc.dram_tensor("bk", [COLS, 1], F32, kind="ExternalInput").ap()
        bv = nc.dram_tensor("bv", [1, COLS], WDT, kind="ExternalInput").ap()
        ones_d = nc.dram_tensor("ones_d", [1, 128], WDT, kind="ExternalInput").ap()
    out = nc.dram_tensor("out", [S, COLS], F32, kind="ExternalOutput").ap()

    with tile.TileContext(nc) as tc:
      for _rep in range(reps):
        with ExitStack() as ctx:
            persist = ctx.enter_context(tc.tile_pool(name="persist", bufs=1))
            wpool = ctx.enter_context(tc.tile_pool(name="wpool", bufs=1))
            xpool = ctx.enter_context(
                tc.tile_pool(
                    name="xpool",
                    bufs=(xbufs if xbufs else (2 if p4 else 3)),
                )
            )
            mpool = ctx.enter_context(tc.tile_pool(name="mpool", bufs=mbufs))
            ppool = ctx.enter_context(tc.tile_pool(name="ppool", bufs=1))
            opool = ctx.enter_context(tc.tile_pool(name="opool", bufs=2))
            upool = ctx.enter_context(tc.tile_pool(name="upool", bufs=2))
            rpool = ctx.enter_context(tc.tile_pool(name="rpool", bufs=2))
            psum = ctx.enter_context(tc.tile_pool(name="psum", bufs=1, space="PSUM"))

            # ---- constants / persistent ----
            ident = persist.tile([128, 128], F32)
            make_identity(nc, ident[:])
            if early:
                bqk_sb = persist.tile([128, 2, 2], F32)
                cw_sb = persist.tile([1, COLS + 128], WDT)
                bv_sb = cw_sb[0:1, 0:COLS]
                ones1 = cw_sb[0:1, COLS : COLS + 128]
            else:
                ones1 = persist.tile([1, 128], WDT)
                bq_sb = persist.tile([128, 2], F32)
                bk_sb = persist.tile([128, 2], F32)
                bv_sb = persist.tile([1, COLS], WDT)

            def load_consts():
                if early:
                    nc.sync.dma_start(
                        bqk_sb[:], bqk.rearrange("(hp p) i -> p hp i", p=128)
                    )
                    nc.sync.dma_start(cw_sb[:], cw[:])
                    return
                nc.sync.dma_start(ones1[:], ones_d[:])
                for hp in range(2):
                    nc.sync.dma_start(
                        bq_sb[:, hp : hp + 1], bq[hp * 128 : hp * 128 + 128, :]
                    )
                    nc.sync.dma_start(
                        bk_sb[:, hp : hp + 1], bk[hp * 128 : hp * 128 + 128, :]
                    )
                nc.sync.dma_start(bv_sb[:], bv[:])

            if not early:
                QT = [persist.tile([128, S], WDT, name=f"QT{p}") for p in range(2)]
                KT = [persist.tile([128, S], WDT, name=f"KT{p}") for p in range(2)]
            Vp = persist.tile([128, JT, LHEADS, 66], PDT)
            nc.gpsimd.memset(Vp[:, :, :, 64:65], 1.0)

            # weights: K is loaded interleaved with the first xT block so the
            # first matmul starts after ~2 chunks; Q/V weights load later.
            wk_sb = wpool.tile([128, HC, COLS], WDT)
            wq_sb = wpool.tile([128, HC, COLS], WDT)
            wv_sb = wpool.tile([128, HC, COLS], WDT)

            def load_w(w_sb, w_dram):
                for hc in range(HC):
                    nc.sync.dma_start(
                        w_sb[:, hc, :], w_dram[hc * 128 : (hc + 1) * 128, :]
                    )

            def load_xt(sb):
                xt = xpool.tile([128, HC, SBLK], WDT, name="xt", tag="xt")
                for hc in range(HC):
                    nc.sync.dma_start(
                        xt[:, hc, :],
                        xT[hc * 128 : (hc + 1) * 128, sb * SBLK : (sb + 1) * SBLK],
                    )
                return xt

            def proj_k(sb, xt):
                sl = slice(sb * SBLK, (sb + 1) * SBLK)
                for hp in range(2):
                    cs = slice(hp * 128, hp * 128 + 128)
                    # pk borrows the scA/scB slots (attention only starts
                    # after K is fully projected anyway)
                    pk = psum.tile(
                        [128, SBLK], F32, name="pk", tag=f"sc{hp}",
                        bufs=(2 if sc_fd512 else 1),
                    )
                    for hc in range(HC):
                        nc.tensor.matmul(
                            pk[:], wk_sb[:, hc, cs], xt[:, hc, :],
                            start=(hc == 0), stop=(hc == HC - 1),
                        )
                    nc.vector.tensor_scalar_add(
                        KT[hp][:, sl], pk[:], bk_sb[:, hp : hp + 1]
                    )

            def proj_q(sb, xt):
                sl = slice(sb * SBLK, (sb + 1) * SBLK)
                for hp in range(2):
                    cs = slice(hp * 128, hp * 128 + 128)
                    pq = psum.tile([128, SBLK], F32, name="pq", tag="pq")
                    for hc in range(HC):
                        nc.tensor.matmul(
                            pq[:], wq_sb[:, hc, cs], xt[:, hc, :],
                            start=(hc == 0), stop=(hc == HC - 1),
                        )
                    nc.vector.tensor_scalar_add(
                        QT[hp][:, sl], pq[:], bq_sb[:, hp : hp + 1]
                    )

            def proj_v(sb, xt):
                for st4 in range(4):
                    st = sb * 4 + st4
                    pv = psum.tile(
                        [128, COLS], F32, name="pv", tag=f"po{st % 2}"
                    )
                    for hc in range(HC):
                        nc.tensor.matmul(
                            pv[:], xt[:, hc, st4 * 128 : st4 * 128 + 128],
                            wv_sb[:, hc, :],
                            start=(hc == 0), stop=False,
                        )
                    nc.tensor.matmul(
                        pv[:], ones1[:], bv_sb[:], start=False, stop=True
                    )
                    nc.vector.tensor_copy(
                        Vp[:, st, :, 0:64],
                        pv.rearrange("p (h d) -> p h d", h=LHEADS),
                    )

            if early:
                # Per-sb K/Q tiles + thunk-interleaved projections so the
                # first exp fires at ~8us instead of ~60us (ACT is the
                # attention-phase roofline; it must start ASAP).
                KTs = [
                    [persist.tile([128, SBLK], WDT, name=f"KTs{p}{sb}")
                     for sb in range(NSB)]
                    for p in range(2)
                ]
                QTs = [
                    [persist.tile([128, SBLK], WDT, name=f"QTs{p}{sb}")
                     for sb in range(NSB)]
                    for p in range(2)
                ]
                xt0 = persist.tile([128, HC, SBLK], WDT, name="xt0")
                xa3 = persist.tile([128, HC, 3 * SBLK], WDT, name="xa3")

                def xsl(hc, lo, hi):
                    if hi <= SBLK:
                        return xt0[:, hc, lo:hi]
                    return xa3[:, hc, lo - SBLK : hi - SBLK]

                scc = [0]

                def enext():
                    t = ["sc0", "sc1", "pq"][scc[0] % 3]
                    scc[0] += 1
                    return t

                # Few, large DMAs: HWDGE costs ~625ns per dma_start
                # regardless of size, so the input stream is issue-bound.
                # Order: K/Q(sb0) inputs, then x for sb1-3 (feeds K-proj
                # thunks just ahead of stage A's j-tile consumption), then
                # masks/V weights (consumed later; ib0/hp0 mask-muls are
                # deferred on DVE to tolerate the late mask arrival).
                nc.sync.dma_start(
                    wk_sb[:], wk.rearrange("(c p) n -> p c n", p=128)
                )
                nc.sync.dma_start(
                    xt0[:, 0:4, :],
                    xT[0:512, 0:SBLK].rearrange("(c p) s -> p c s", p=128),
                )
                nc.sync.dma_start(
                    wq_sb[:], wq.rearrange("(c p) n -> p c n", p=128)
                )
                nc.sync.dma_start(
                    xt0[:, 4:HC, :],
                    xT[512:H, 0:SBLK].rearrange("(c p) s -> p c s", p=128),
                )
                load_consts()
                for sb in range(1, NSB):
                    ssl = slice(sb * SBLK, (sb + 1) * SBLK)
                    nc.sync.dma_start(
                        xa3[:, :, (sb - 1) * SBLK : sb * SBLK],
                        xT[:, ssl].rearrange("(c p) s -> p c s", p=128),
                    )
                mts = {}

                def load_mask(ib):
                    mts[ib] = mpool.tile([128, JT, SBLK], PDT, name="mt")
                    nc.sync.dma_start(
                        mts[ib][:],
                        maskp[:, ib * SBLK : (ib + 1) * SBLK].rearrange(
                            "(t p) i -> p t i", p=128
                        ),
                    )

                load_mask(0)
                nc.sync.dma_start(
                    wv_sb[:], wv.rearrange("(c p) n -> p c n", p=128)
                )
                load_mask(1)

                def proj_kq_sb(w_sb, qk, dst, hp, sb):
                    cs = slice(hp * 128, hp * 128 + 128)
                    pk = psum.tile([128, SBLK], F32, name="pk", tag=enext())
                    for hc in range(HC):
                        nc.tensor.matmul(
                            pk[:], w_sb[:, hc, cs],
                            xsl(hc, sb * SBLK, (sb + 1) * SBLK),
                            start=(hc == 0), stop=(hc == HC - 1),
                        )
                    nc.vector.tensor_scalar_add(
                        dst[hp][sb][:], pk[:], bqk_sb[:, hp, qk : qk + 1]
                    )

                def proj_v_st(st):
                    pv = psum.tile([128, COLS], F32, name="pv", tag=enext())
                    for hc in range(HC):
                        nc.tensor.matmul(
                            pv[:], xsl(hc, st * 128, (st + 1) * 128),
                            wv_sb[:, hc, :],
                            start=(hc == 0), stop=False,
                        )
                    nc.tensor.matmul(
                        pv[:], ones1, bv_sb, start=False, stop=True
                    )
                    nc.vector.tensor_copy(
                        Vp[:, st, :, 0:64],
                        pv.rearrange("p (h d) -> p h d", h=LHEADS),
                    )

                # hp0 first: stage A (ib0, hp0) only needs the hp0 halves.
                proj_kq_sb(wk_sb, 1, KTs, 0, 0)
                proj_kq_sb(wq_sb, 0, QTs, 0, 0)
                proj_kq_sb(wk_sb, 1, KTs, 1, 0)
                proj_kq_sb(wq_sb, 0, QTs, 1, 0)

                thunks = []
                for sb in range(1, NSB):
                    for hp in range(2):
                        thunks.append(
                            lambda sb=sb, hp=hp: proj_kq_sb(
                                wk_sb, 1, KTs, hp, sb
                            )
                        )
                for st in range(JT):
                    thunks.append(lambda st=st: proj_v_st(st))
                for sb in range(1, NSB):
                    for hp in range(2):
                        thunks.append(
                            lambda sb=sb, hp=hp: proj_kq_sb(
                                wq_sb, 0, QTs, hp, sb
                            )
                        )

                deferred = []
                for ib in range(NSB):
                    if ib not in mts:
                        load_mask(ib)
                    if ib >= 1 and ib + 1 < NSB and ib + 1 not in mts:
                        load_mask(ib + 1)
                    mt = mts[ib]
                    isl = slice(ib * SBLK, (ib + 1) * SBLK)
                    for hp in range(2):
                        outt = opool.tile(
                            [128, 4, 128], F32, name=f"outt{hp}",
                            tag=f"outt{hp}",
                        )
                        P2h = [
                            ppool.tile(
                                [128, JT, SBLK], PDT, name=f"P{hl}",
                                tag=f"P{hp}{hl}",
                            )
                            for hl in range(2)
                        ]
                        po2 = [
                            psum.tile(
                                [65, SBLK], F32, name=f"po{hl}", tag=f"po{hl}"
                            )
                            for hl in range(2)
                        ]
                        def stage_b(jg_):
                            for hl in range(2):
                                h = hp * 2 + hl
                                for jj in range(2):
                                    jt_ = jg_ * 2 + jj
                                    nc.tensor.matmul(
                                        po2[hl][:],
                                        Vp[:, jt_, h, 0:65],
                                        P2h[hl][:, jt_, :],
                                        start=(jt_ == 0),
                                        stop=(jt_ == JT - 1),
                                        skip_group_check=True,
                                    )

                        for jg in range(JG):
                            for hl in range(2):
                                ps = psum.tile(
                                    [128, 2, SBLK], F32, name=f"sc{hl}",
                                    tag=enext(),
                                )
                                rows = slice(hl * 64, hl * 64 + 64)
                                for jj in range(2):
                                    jt_ = jg * 2 + jj
                                    nc.tensor.matmul(
                                        ps[:, jj, :],
                                        KTs[hp][jt_ // 4][
                                            rows,
                                            (jt_ % 4) * 128 : (jt_ % 4 + 1) * 128,
                                        ],
                                        QTs[hp][ib][rows, :],
                                        start=True,
                                        stop=True,
                                    )
                                nc.scalar.activation(
                                    P2h[hl][:, jg * 2 : jg * 2 + 2, :],
                                    ps[:],
                                    mybir.ActivationFunctionType.Exp,
                                    scale=0.125,
                                )

                                def mmul(hl=hl, jg=jg):
                                    nc.vector.tensor_mul(
                                        P2h[hl][:, jg * 2 : jg * 2 + 2, :],
                                        P2h[hl][:, jg * 2 : jg * 2 + 2, :],
                                        mt[:, jg * 2 : jg * 2 + 2, :],
                                    )

                                # ib0/hp0 masks arrive after the x/w DMAs;
                                # defer their muls so the in-order DVE queue
                                # isn't head-of-line blocked on the mask DMA.
                                if ib == 0 and hp == 0:
                                    deferred.append(mmul)
                                else:
                                    mmul()
                                # 2 thunks/slot while K/V remain: every
                                # K and V projection must be EMITTED before
                                # the first stage_b emission (slot 16) —
                                # emission order IS the dependency order.
                                for _ in range(2 if len(thunks) > 6 else 1):
                                    if thunks:
                                        thunks.pop(0)()
                            if ib == NSB - 1 and hp == 1 and jg >= 1:
                                stage_b(jg - 1)
                        for dm in deferred:
                            dm()
                        deferred = []
                        if ib == NSB - 1 and hp == 1:
                            stage_b(JG - 1)
                        else:
                            for jg_ in range(JG):
                                stage_b(jg_)
                        for hl in range(2):
                            po = po2[hl]
                            u = upool.tile([65, SBLK], F32, name="u")
                            nc.vector.tensor_copy(u[:], po[:])
                            pt = psum.tile(
                                [128, 4, 65], F32, name="pt",
                                tag=(enext() if ib == NSB - 1 and hp == 1
                                     else "po0"),
                            )
                            for c in range(4):
                                nc.tensor.transpose(
                                    pt[:, c, :],
                                    u[:, c * 128 : (c + 1) * 128],
                                    ident[0:65, 0:65],
                                )
                            rec = rpool.tile([128, 4], F32, name="rec")
                            nc.vector.reciprocal(rec[:], pt[:, :, 64])
                            for c in range(4):
                                nc.vector.tensor_scalar_mul(
                                    outt[:, c, hl * 64 : hl * 64 + 64],
                                    pt[:, c, 0:64],
                                    rec[:, c : c + 1],
                                )
                        nc.sync.dma_start(
                            out[isl, hp * 128 : (hp + 1) * 128].rearrange(
                                "(c p) n -> p c n", p=128
                            ),
                            outt[:],
                        )
                continue

            # K pass first (attention stage A gates on full KT), then Q/V.
            # First block: interleave wk-chunk and xt-chunk DMAs so the
            # leading matmul's operands land after ~400KB of traffic.
            xt0 = xpool.tile([128, HC, SBLK], WDT, name="xt", tag="xt")
            for hc in range(HC):
                nc.sync.dma_start(
                    wk_sb[:, hc, :], wk[hc * 128 : (hc + 1) * 128, :]
                )
                nc.sync.dma_start(
                    xt0[:, hc, :], xT[hc * 128 : (hc + 1) * 128, 0:SBLK]
                )
            load_consts()
            if single_pass:
                load_w(wq_sb, wq)
                load_w(wv_sb, wv)
                proj_k(0, xt0)
                proj_q(0, xt0)
                proj_v(0, xt0)
                for sb in range(1, NSB):
                    xt = load_xt(sb)
                    proj_k(sb, xt)
                    proj_q(sb, xt)
                    proj_v(sb, xt)
            else:
                proj_k(0, xt0)
                for sb in range(1, NSB):
                    xt = load_xt(sb)
                    if sb == 1:
                        load_w(wq_sb, wq)
                    elif sb == 2:
                        load_w(wv_sb, wv)
                    proj_k(sb, xt)
                for sb in range(NSB):
                    xt = load_xt(sb)
                    proj_q(sb, xt)
                    proj_v(sb, xt)

            # ---- attention ----
            if zz:
                # software-pipelined: P split into 8 half-buffers
                # (hp x hl x half); stage A/B zig-zag so ACT (exp) never
                # starves while PE runs stage B / projections.
                HJT = JT // 2  # j-tiles per half

                def emit_A(ib, hp, half, Ph):
                    isl = slice(ib * SBLK, (ib + 1) * SBLK)
                    mt = mts[ib]
                    for g in range(HJT // 2):
                        for hl in range(2):
                            ps = psum.tile(
                                [128, 2, SBLK], F32, name=f"sc{hl}", tag=f"sc{hl}"
                            )
                            rows = slice(hl * 64, hl * 64 + 64)
                            for jj in range(2):
                                jt_ = half * HJT + g * 2 + jj
                                nc.tensor.matmul(
                                    ps[:, jj, :],
                                    KT[hp][rows, jt_ * 128 : jt_ * 128 + 128],
                                    QT[hp][rows, isl],
                                    start=True,
                                    stop=True,
                                )
                            gsl = slice(g * 2, g * 2 + 2)
                            msl = slice(half * HJT + g * 2, half * HJT + g * 2 + 2)
                            nc.scalar.activation(
                                Ph[hl][:, gsl, :],
                                ps[:],
                                mybir.ActivationFunctionType.Exp,
                                scale=0.125,
                            )
                            nc.vector.tensor_mul(
                                Ph[hl][:, gsl, :], Ph[hl][:, gsl, :], mt[:, msl, :]
                            )

                def emit_B(ib, hp, half, Ph, po2):
                    for hl in range(2):
                        h = hp * 2 + hl
                        for g in range(HJT):
                            jt_ = half * HJT + g
                            nc.tensor.matmul(
                                po2[hl][:],
                                Vp[:, jt_, h, 0:65],
                                Ph[hl][:, g, :],
                                start=(jt_ == 0),
                                stop=(jt_ == JT - 1),
                                skip_group_check=True,
                            )

                def emit_epi(ib, hp, outt, po2):
                    for hl in range(2):
                        h = hp * 2 + hl
                        po = po2[hl]
                        u = upool.tile([65, SBLK], F32, name="u")
                        nc.vector.tensor_copy(u[:], po[:])
                        pt = psum.tile([128, 4, 65], F32, name="pt", tag="pt")
                        for c in range(4):
                            nc.tensor.transpose(
                                pt[:, c, :],
                                u[:, c * 128 : (c + 1) * 128],
                                ident[0:65, 0:65],
                            )
                        rec = rpool.tile([128, 4], F32, name="rec")
                        nc.vector.reciprocal(rec[:], pt[:, :, 64])
                        for c in range(4):
                            nc.vector.tensor_scalar_mul(
                                outt[:, c, h * 64 : h * 64 + 64],
                                pt[:, c, 0:64],
                                rec[:, c : c + 1],
                            )

                def new_P(hp):
                    return [
                        [
                            ppool.tile(
                                [128, HJT, SBLK], PDT,
                                name=f"P{hp}{hl}{hf}", tag=f"P{hp}{hl}{hf}",
                            )
                            for hf in range(2)
                        ]
                        for hl in range(2)
                    ]

                def P_half(P, hf):
                    return [P[0][hf], P[1][hf]]

                mts = {}
                for ib in range(NSB):
                    isl = slice(ib * SBLK, (ib + 1) * SBLK)
                    mts[ib] = mpool.tile([128, JT, SBLK], PDT, name="mt")
                    nc.sync.dma_start(
                        mts[ib][:],
                        maskp[:, isl].rearrange("(t p) i -> p t i", p=128),
                    )
                    outt = opool.tile([128, 4, COLS], F32, name="outt")
                    P0 = new_P(0)
                    P1 = new_P(1)
                    po0 = [
                        psum.tile([65, SBLK], F32, name=f"po{hl}", tag=f"po{hl}")
                        for hl in range(2)
                    ]
                    emit_A(ib, 0, 0, P_half(P0, 0))
                    emit_A(ib, 0, 1, P_half(P0, 1))
                    emit_A(ib, 1, 0, P_half(P1, 0))
                    emit_B(ib, 0, 0, P_half(P0, 0), po0)
                    emit_A(ib, 1, 1, P_half(P1, 1))
                    emit_B(ib, 0, 1, P_half(P0, 1), po0)
                    emit_epi(ib, 0, outt, po0)
                    po1 = [
                        psum.tile([65, SBLK], F32, name=f"qo{hl}", tag=f"po{hl}")
                        for hl in range(2)
                    ]
                    emit_B(ib, 1, 0, P_half(P1, 0), po1)
                    emit_B(ib, 1, 1, P_half(P1, 1), po1)
                    emit_epi(ib, 1, outt, po1)
                    nc.sync.dma_start(
                        out[isl, :].rearrange("(c p) n -> p c n", p=128), outt[:]
                    )
            else:
                scc = [0]

                def next_sc(hl):
                    if not sc3:
                        return f"sc{hl}"
                    t = ["sc0", "sc1", "pq"][scc[0] % 3]
                    scc[0] += 1
                    return t

                for ib in range(NSB):
                    isl = slice(ib * SBLK, (ib + 1) * SBLK)
                    mt = mpool.tile([128, JT, SBLK], PDT, name="mt")
                    nc.sync.dma_start(
                        mt[:], maskp[:, isl].rearrange("(t p) i -> p t i", p=128)
                    )
                    if not dma2:
                        outt = opool.tile([128, 4, COLS], F32, name="outt")
                    for hp in range(2):
                        if dma2:
                            outt = opool.tile(
                                [128, 4, 128], F32, name=f"outt{hp}",
                                tag=f"outt{hp}",
                            )
                        ptag = f"P{hp}" if p4 else "P"
                        P2h = [
                            ppool.tile(
                                [128, JT, SBLK], PDT, name=f"P{hl}",
                                tag=f"{ptag}{hl}",
                            )
                            for hl in range(2)
                        ]
                        po2 = [
                            psum.tile([65, SBLK], F32, name=f"po{hl}", tag=f"po{hl}")
                            for hl in range(2)
                        ]

                        def stage_b(jg_done):
                            for hl in range(2):
                                h = hp * 2 + hl
                                for jj in range(2):
                                    jt_ = jg_done * 2 + jj
                                    nc.tensor.matmul(
                                        po2[hl][:],
                                        Vp[:, jt_, h, 0:65],
                                        P2h[hl][:, jt_, :],
                                        start=(jt_ == 0),
                                        stop=(jt_ == JT - 1),
                                        skip_group_check=True,
                                    )

                        if sc_fd512:
                            for jt_ in range(JT):
                                for hl in range(2):
                                    ps = psum.tile(
                                        [128, SBLK], F32, name=f"sc{hl}",
                                        tag=f"sc{hl}", bufs=2,
                                    )
                                    rows = slice(hl * 64, hl * 64 + 64)
                                    nc.tensor.matmul(
                                        ps[:],
                                        KT[hp][rows, jt_ * 128 : jt_ * 128 + 128],
                                        QT[hp][rows, isl],
                                        start=True,
                                        stop=True,
                                    )
                                    nc.scalar.activation(
                                        P2h[hl][:, jt_, :],
                                        ps[:],
                                        mybir.ActivationFunctionType.Exp,
                                        scale=0.125,
                                    )
                                    nc.vector.tensor_mul(
                                        P2h[hl][:, jt_, :],
                                        P2h[hl][:, jt_, :],
                                        mt[:, jt_, :],
                                    )
                        else:
                          for jg in range(JG):
                            for hl in range(2):
                                ps = psum.tile(
                                    [128, 2, SBLK], F32, name=f"sc{hl}",
                                    tag=next_sc(hl),
                                )
                                rows = slice(hl * 64, hl * 64 + 64)
                                for jj in range(2):
                                    jt_ = jg * 2 + jj
                                    nc.tensor.matmul(
                                        ps[:, jj, :],
                                        KT[hp][rows, jt_ * 128 : jt_ * 128 + 128],
                                        QT[hp][rows, isl],
                                        start=True,
                                        stop=True,
                                    )
                                gsl = slice(jg * 2, jg * 2 + 2)
                                nc.scalar.activation(
                                    P2h[hl][:, gsl, :],
                                    ps[:],
                                    mybir.ActivationFunctionType.Exp,
                                    scale=0.125,
                                )
                                eng = (
                                    nc.gpsimd
                                    if gsplit and (jg % gsplit == gsplit - 1)
                                    else nc.vector
                                )
                                eng.tensor_mul(
                                    P2h[hl][:, gsl, :], P2h[hl][:, gsl, :], mt[:, gsl, :]
                                )
                            if interleave_b and jg >= 1:
                                stage_b(jg - 1)
                        if interleave_b:
                            stage_b(JG - 1)
                        else:
                            for jg_ in range(JG):
                                stage_b(jg_)

                        for hl in range(2):
                            h = hp * 2 + hl
                            po = po2[hl]
                            u = upool.tile([65, SBLK], F32, name="u")
                            nc.vector.tensor_copy(u[:], po[:])
                            pt = psum.tile(
                                [128, 4, 65], F32, name="pt",
                                tag=("po0" if sc3 else "pt"),
                            )
                            for c in range(4):
                                nc.tensor.transpose(
                                    pt[:, c, :],
                                    u[:, c * 128 : (c + 1) * 128],
                                    ident[0:65, 0:65],
                                )
                            rec = rpool.tile([128, 4], F32, name="rec")
                            nc.vector.reciprocal(rec[:], pt[:, :, 64])
                            oc = hl * 64 if dma2 else h * 64
                            for c in range(4):
                                nc.vector.tensor_scalar_mul(
                                    outt[:, c, oc : oc + 64],
                                    pt[:, c, 0:64],
                                    rec[:, c : c + 1],
                                )
                        if dma2:
                            nc.sync.dma_start(
                                out[isl, hp * 128 : (hp + 1) * 128].rearrange(
                                    "(c p) n -> p c n", p=128
                                ),
                                outt[:],
                            )
                    if not dma2:
                        nc.sync.dma_start(
                            out[isl, :].rearrange("(c p) n -> p c n", p=128),
                            outt[:],
                        )

    nc.compile()
    return nc


_NC_CACHE = []


BUILD_KW = dict(fp16=True, mbufs=3, early=True)


def get_nc():
    if not _NC_CACHE:
        _NC_CACHE.append(build_program(**BUILD_KW))
    return _NC_CACHE[0]


def make_in_maps(x, attn_mask, Wq, bq, Wk, bk, Wv, bv, fp16=False):
    wdt = np.float16 if fp16 else np.float32
    pdt = np.float16 if fp16 else ml_dtypes.bfloat16
    x = np.asarray(x, dtype=np.float32)
    attn_mask = np.asarray(attn_mask)
    Wq, Wk, Wv = (np.asarray(w, dtype=np.float32) for w in (Wq, Wk, Wv))
    bq, bk, bv = (np.asarray(b_, dtype=np.float32) for b_ in (bq, bk, bv))

    in_maps = []
    for core in range(NCORES):
        b = core // CORES_PER_B
        hg = core % CORES_PER_B
        cs = slice(hg * COLS, (hg + 1) * COLS)
        mp = (1 - attn_mask[b].T).astype(pdt)
        cw = np.concatenate(
            [bv[cs].astype(np.float32), np.ones(128, np.float32)]
        )[None, :]
        in_maps.append(
            {
                "xT": np.ascontiguousarray(x[b].T.astype(wdt)),
                "maskp": np.ascontiguousarray(mp),
                "wq": np.ascontiguousarray(Wq[:, cs].astype(wdt)),
                "wk": np.ascontiguousarray(Wk[:, cs].astype(wdt)),
                "wv": np.ascontiguousarray(Wv[:, cs].astype(wdt)),
                "bqk": np.ascontiguousarray(
                    np.stack([bq[cs], bk[cs]], axis=1).astype(np.float32)
                ),
                "cw": np.ascontiguousarray(cw.astype(wdt)),
            }
        )
    return in_maps


def assemble(results):
    out = np.empty((B, S, H), np.float32)
    for core in range(NCORES):
        b = core // CORES_PER_B
        hg = core % CORES_PER_B
        out[b, :, hg * COLS : (hg + 1) * COLS] = results[core]["out"]
    return out


USE_FP16 = True


def kernel(x, attn_mask, Wq, bq, Wk, bk, Wv, bv):
    nc = get_nc()
    in_maps = make_in_maps(x, attn_mask, Wq, bq, Wk, bk, Wv, bv, fp16=USE_FP16)
    res = run_bass_kernel_spmd(nc, in_maps, list(range(NCORES)))
    return assemble(res.results)

